# revision 7
# baseline (speedup 1.0000x reference)
"""Trainium2 Bass kernel for nn_CausalSelfAttention_24034636988727 (B=1,T=4096,C=768,H=12).

Math identity used: denom = cumsum(qn@kn^T, axis=-1) = qn @ cumsum(kn, axis=0)^T,
so the TxT cumsum collapses to a [T,hd] prefix-sum plus a second matmul and the
whole attention stays on-chip (no TxT traffic to HBM).

Sharding (8 cores, two SPMD launches, full I/O in host numpy):
  L1: T-sharded qkv projection (q,k fp32; v->f32r), l2-normalize q,k,
      emit transposed [c',t] q,k plus f32r-rounded copies (and q residual for
      a 3-term f32r "split" den matmul at ~fp32 accuracy, 3 cyc/row vs 4).
  host: concatenate shards (data movement only).
  L2: q-block sharded. Per head: prefix-scan kn^T -> S (GPSIMD);
      num=qnr@knr^T (f32r, 1 cyc/row); den=Sr@qnr+Sr@qe+Se@qnr (f32r x3);
      att=num*recip(max(den,1e-6)) via DVE clamp + ACT reciprocal + DVE mult;
      y^T accumulated on PE (f32r); output projection (f32r) + biases.
"""

import sys

sys.path.insert(0, "/opt/trn_rl_repo")

import numpy as np

import concourse.bass as bass
import concourse.mybir as mybir
import concourse.tile as tile
from concourse.tile import ScopedClock
from concourse.bass_utils import run_bass_kernel_spmd

N_CORES = 8
T = 4096
C = 768
H = 12
HD = 64
TS = T // N_CORES        # 512 q rows per core
HALF = T // 2            # k-halves per head in L2 (SBUF footprint)
NKC = T // 128           # 32 k-chunks per head
NCH = C // 128           # 6 contraction chunks
f32 = mybir.dt.float32
f32r = mybir.dt.float32r
AF = mybir.ActivationFunctionType
ALU = mybir.AluOpType

EPS_NORM = 1e-12
EPS_DENOM = 1e-6

# tuning knobs
SCAN_ON_GPSIMD = False  # Pool TensorScalarPtr rejected by this walrus
DEN_SPLIT3 = True    # den via 3 f32r matmuls instead of 1 plain-fp32 matmul
CLAMP_SPLIT = 1.0    # fraction of k-chunks whose clamp runs on DVE (rest: ACT relu path)


class TC(tile.TileContext):
    """TileContext whose final drain spreads its waits over several SP drains
    (this walrus build allows only one sync wait per instruction)."""

    def _drain_and_barrier(self, tick_clock, wait_clock):
        nc = self.nc
        probe = nc.sync.drain()
        wait_clock.add_sem_waits(probe.ins, ScopedClock({None: tick_clock.global_clock}))
        waits = list(probe.ins.sync_info.on_wait)
        probe.ins.sync_info.on_wait = waits[:1]
        for w in waits[1:]:
            n2 = nc.sync.drain()
            si = n2.ins.sync_info
            if si is None:
                si = mybir.SyncInfo(on_wait=[], on_update=[])
                n2.ins.sync_info = si
            si.on_wait = [w]
        nc.all_engine_barrier()
        assert self.sems is not None
        popped = nc._tile_sem_poison_stack.pop()
        assert popped is self._sem_poison
        nc.clear_and_free_semaphores(list(self.sems.allocated().values()))
        nc.all_engine_barrier()


def legalize_waits(nc):
    """This walrus accepts at most one sync wait per instruction; hoist extra
    waits onto same-engine NoOps placed immediately before the instruction."""
    for f in nc.m.functions:
        for bb in f.blocks:
            out = []
            changed = False
            for ins in list(bb.instructions):
                si = ins.sync_info
                ow = list(si.on_wait) if (si is not None and si.on_wait) else []
                if len(ow) > 1:
                    for j, w in enumerate(ow[:-1]):
                        out.append(
                            mybir.InstNoOp(
                                name=f"{ins.name}-lw{j}",
                                engine=ins.engine,
                                ins=[],
                                outs=[],
                                sync_info=mybir.SyncInfo(on_wait=[w], on_update=[]),
                            )
                        )
                    si.on_wait = [ow[-1]]
                    ins.sync_info = si
                    changed = True
                out.append(ins)
            if changed:
                bb.instructions = out


def act_reciprocal(nc, out_ap, in_ap, bias=0.0):
    """1/(x+bias) on the Activation engine (direct emission; the bass wrapper
    blanket-bans Reciprocal, but measured accuracy here is ~1e-5 max rel err)."""
    return nc.scalar.add_instruction(
        mybir.InstActivation(
            name=nc.get_next_instruction_name(),
            func=AF.Reciprocal,
            ins=[
                nc.scalar.lower_ap(in_ap),
                mybir.ImmediateValue(dtype=f32, value=float(bias)),
                mybir.ImmediateValue(dtype=f32, value=1.0),
                mybir.ImmediateValue(dtype=f32, value=0.0),
            ],
            outs=[nc.scalar.lower_ap(out_ap)],
        )
    )


def build_l1():
    nc = bass.Bass("TRN2", target_bir_lowering=False, debug=False)
    xT = nc.dram_tensor("xT", [C, TS], f32, kind="ExternalInput")
    w_qk = nc.dram_tensor("w_qk", [C, 2 * C], f32, kind="ExternalInput")
    w_v = nc.dram_tensor("w_v", [C, C], f32, kind="ExternalInput")
    b_qk = nc.dram_tensor("b_qk", [1, 2 * C], f32, kind="ExternalInput")
    b_v = nc.dram_tensor("b_v", [1, C], f32, kind="ExternalInput")
    kn_o = nc.dram_tensor("kn_o", [C, TS], f32, kind="ExternalOutput")
    knr_o = nc.dram_tensor("knr_o", [C, TS], f32r, kind="ExternalOutput")
    qn_o = nc.dram_tensor("qn_o", [C, TS], f32, kind="ExternalOutput")
    qnr_o = nc.dram_tensor("qnr_o", [C, TS], f32r, kind="ExternalOutput")
    qe_o = nc.dram_tensor("qe_o", [C, TS], f32r, kind="ExternalOutput")
    v_o = nc.dram_tensor("v_o", [TS, C], f32r, kind="ExternalOutput")

    with TC(nc) as tc:
        with (
            tc.tile_pool(name="inp", bufs=1) as inp,
            tc.tile_pool(name="proj", bufs=1) as proj,
            tc.tile_pool(name="outw", bufs=3) as outw,
            tc.tile_pool(name="work", bufs=2) as work,
            tc.tile_pool(name="ps_a", bufs=2, space="PSUM") as ps_a,
            tc.tile_pool(name="ps_b", bufs=2, space="PSUM") as ps_b,
            tc.tile_pool(name="ps_c", bufs=2, space="PSUM") as ps_c,
        ):
            xt_sb = []
            for ci in range(NCH):
                t_ = inp.tile([128, TS], f32, tag=f"xt{ci}")
                nc.sync.dma_start(t_[:], xT[ci * 128:(ci + 1) * 128, :])
                xt_sb.append(t_)
            wqk_sb = []
            for ci in range(NCH):
                t_ = inp.tile([128, 2 * C], f32, tag=f"wqk{ci}")
                nc.sync.dma_start(t_[:], w_qk[ci * 128:(ci + 1) * 128, :])
                wqk_sb.append(t_)
            wv_sb = []
            for ci in range(NCH):
                t_ = inp.tile([128, C], f32, tag=f"wv{ci}")
                nc.sync.dma_start(t_[:], w_v[ci * 128:(ci + 1) * 128, :])
                wv_sb.append(t_)
            bqk_sb = inp.tile([1, 2 * C], f32, tag="bqk")
            nc.sync.dma_start(bqk_sb[:], b_qk[:])
            bv_sb = inp.tile([1, C], f32, tag="bv")
            nc.sync.dma_start(bv_sb[:], b_v[:])
            ones_r = inp.tile([12, TS], f32, tag="ones_r")
            nc.vector.memset(ones_r[:], 1.0)
            ones_c = inp.tile([128, 1], f32, tag="ones_c")
            nc.vector.memset(ones_c[:], 1.0)

            # q,k projection, transposed layout [c', t] (plain fp32 matmuls)
            qkT = []
            for j in range(12):
                ps = ps_a.tile([128, TS], f32, tag="proj_ps")
                for ci in range(NCH):
                    nc.tensor.matmul(
                        ps[:], wqk_sb[ci][:, j * 128:(j + 1) * 128], xt_sb[ci][:],
                        start=(ci == 0), stop=False)
                nc.tensor.matmul(
                    ps[:], bqk_sb[0:1, j * 128:(j + 1) * 128], ones_r[0:1, :],
                    start=False, stop=True)
                t_ = proj.tile([128, TS], f32, tag=f"qkT{j}")
                nc.scalar.copy(t_[:], ps[:])
                qkT.append(t_)

            # v projection, natural layout [t, c'] (fp32 matmul, f32r-rounded out)
            for tt in range(TS // 128):
                t_ = outw.tile([128, C], f32r, tag="v_nat")
                for c0, cn in ((0, 512), (512, 256)):
                    ps = ps_b.tile([128, 512], f32, tag="v_ps")
                    for ci in range(NCH):
                        nc.tensor.matmul(
                            ps[:, :cn],
                            xt_sb[ci][:, tt * 128:(tt + 1) * 128],
                            wv_sb[ci][:, c0:c0 + cn],
                            start=(ci == 0), stop=False)
                    nc.tensor.matmul(
                        ps[:, :cn], ones_r[0:1, 0:128], bv_sb[0:1, c0:c0 + cn],
                        start=False, stop=True)
                    nc.vector.tensor_copy(t_[:, c0:c0 + cn], ps[:, :cn])
                nc.sync.dma_start(v_o[tt * 128:(tt + 1) * 128, :], t_[:])

            # per-head l2 norms (sumsq over 64 partition rows via ones-matmul),
            # then normalize via ones-outer-product broadcast; round; residual.
            outs = {0: (qn_o, qnr_o), 1: (kn_o, knr_o)}
            for qk in range(2):  # 0: q, 1: k
                o_f32, o_f32r = outs[qk]
                for j in range(6):
                    sq = work.tile([128, TS], f32, tag="sq")
                    nc.scalar.square(sq[:], qkT[qk * 6 + j][:])
                    nrm_t = outw.tile([128, TS], f32, tag="nrmd")
                    rnd_t = outw.tile([128, TS], f32r, tag="rndd")
                    for h2 in range(2):
                        ps1 = ps_c.tile([1, TS], f32, tag="red_ps")
                        nc.tensor.matmul(
                            ps1[:], ones_c[h2 * 64:(h2 + 1) * 64, :],
                            sq[h2 * 64:(h2 + 1) * 64, :], start=True, stop=True)
                        sn = work.tile([1, TS], f32, tag="sn")
                        nc.scalar.sqrt(sn[:], ps1[:])
                        nc.vector.tensor_scalar_max(sn[:], sn[:], EPS_NORM)
                        rn = work.tile([1, TS], f32, tag="rn")
                        act_reciprocal(nc, rn[:], sn[:])
                        psb = ps_c.tile([64, TS], f32, tag="bcast_ps")
                        nc.tensor.matmul(
                            psb[:], ones_r[0:1, 0:64], rn[:],
                            start=True, stop=True)
                        nc.vector.scalar_tensor_tensor(
                            nrm_t[h2 * 64:(h2 + 1) * 64, :], psb[:], 1.0,
                            qkT[qk * 6 + j][h2 * 64:(h2 + 1) * 64, :],
                            ALU.mult, ALU.mult)
                    nc.vector.tensor_copy(rnd_t[:], nrm_t[:])
                    nc.sync.dma_start(o_f32[j * 128:(j + 1) * 128, :], nrm_t[:])
                    nc.sync.dma_start(o_f32r[j * 128:(j + 1) * 128, :], rnd_t[:])
                    if qk == 0 and DEN_SPLIT3:
                        qe_t = outw.tile([128, TS], f32r, tag="qe")
                        nc.vector.tensor_tensor(
                            qe_t[:], nrm_t[:], rnd_t[:].bitcast(f32), ALU.subtract)
                        nc.sync.dma_start(qe_o[j * 128:(j + 1) * 128, :], qe_t[:])
    legalize_waits(nc)
    return nc


def build_l2():
    nc = bass.Bass("TRN2", target_bir_lowering=False, debug=False)
    kn_i = nc.dram_tensor("kn_i", [C, T], f32, kind="ExternalInput")
    knr_i = nc.dram_tensor("knr_i", [C, T], f32r, kind="ExternalInput")
    qn_i = nc.dram_tensor("qn_i", [C, TS], f32, kind="ExternalInput")
    qnr_i = nc.dram_tensor("qnr_i", [C, TS], f32r, kind="ExternalInput")
    qe_i = nc.dram_tensor("qe_i", [C, TS], f32r, kind="ExternalInput")
    v_i = nc.dram_tensor("v_i", [T, C], f32r, kind="ExternalInput")
    w_proj = nc.dram_tensor("w_proj", [C, C], f32, kind="ExternalInput")
    b_proj = nc.dram_tensor("b_proj", [1, C], f32, kind="ExternalInput")
    out_o = nc.dram_tensor("out_o", [TS, C], f32, kind="ExternalOutput")

    NH = HALF // 128  # 16 k-chunks per half

    with TC(nc) as tc:
        with (
            tc.tile_pool(name="inp", bufs=1) as inp,
            tc.tile_pool(name="qh", bufs=2) as qh,
            tc.tile_pool(name="kh", bufs=2) as kh,
            tc.tile_pool(name="ew", bufs=3) as ew,
            tc.tile_pool(name="ps_nd", bufs=2, space="PSUM") as ps_nd,
            tc.tile_pool(name="ps_y", bufs=2, space="PSUM") as ps_y,
        ):
            ones_r = inp.tile([1, 128], f32, tag="ones_r")
            nc.vector.memset(ones_r[:], 1.0)
            negeps = inp.tile([128, 1], f32, tag="negeps")
            nc.vector.memset(negeps[:], -EPS_DENOM)
            wp_sb = []
            for ci in range(NCH):
                tf_ = inp.tile([128, C], f32, tag="wp_tmp")
                nc.sync.dma_start(tf_[:], w_proj[ci * 128:(ci + 1) * 128, :])
                wr = inp.tile([128, C], f32r, tag=f"wpr{ci}")
                nc.vector.tensor_copy(wr[:], tf_[:])
                wp_sb.append(wr)
            bp_sb = inp.tile([1, C], f32, tag="bp")
            nc.sync.dma_start(bp_sb[:], b_proj[:])
            yT = []
            for ci in range(NCH):
                yt_t = inp.tile([128, TS], f32r, tag=f"yT{ci}")
                yT.append(yt_t)

            for h in range(H):
                hs = slice(h * 64, (h + 1) * 64)
                qnr_h = qh.tile([64, TS], f32r, tag="qnr_h")
                nc.sync.dma_start(qnr_h[:], qnr_i[hs, :])
                if DEN_SPLIT3:
                    qe_h = qh.tile([64, TS], f32r, tag="qe_h")
                    nc.sync.dma_start(qe_h[:], qe_i[hs, :])
                else:
                    qn_h = qh.tile([64, TS], f32, tag="qn_h")
                    nc.sync.dma_start(qn_h[:], qn_i[hs, :])
                v_h = qh.tile([128, NKC, 64], f32r, tag="v_h")
                nc.sync.dma_start(
                    v_h[:], v_i[:, hs].rearrange("(c p) d -> p c d", p=128))

                y_ps = ps_y.tile([64, TS], f32, tag="y_ps")
                prev_S = None
                for half in range(2):
                    hsl = slice(half * HALF, (half + 1) * HALF)
                    kn_hh = kh.tile([64, HALF], f32, tag="kn_h")
                    nc.sync.dma_start(kn_hh[:], kn_i[hs, hsl])
                    knr_hh = kh.tile([64, HALF], f32r, tag="knr_h")
                    nc.sync.dma_start(knr_hh[:], knr_i[hs, hsl])
                    S_hh = kh.tile([64, HALF], f32, tag="S_h")
                    init = 0.0 if half == 0 else prev_S[:, HALF - 1:HALF]
                    eng = nc.gpsimd if SCAN_ON_GPSIMD else nc.vector
                    eng.tensor_tensor_scan(
                        S_hh[:], kn_hh[:], kn_hh[:], init, ALU.add, ALU.bypass)
                    prev_S = S_hh
                    if DEN_SPLIT3:
                        Sr_hh = kh.tile([64, HALF], f32r, tag="Sr_h")
                        nc.vector.tensor_copy(Sr_hh[:], S_hh[:])
                        Se_hh = kh.tile([64, HALF], f32r, tag="Se_h")
                        nc.vector.tensor_tensor(
                            Se_hh[:], S_hh[:], Sr_hh[:].bitcast(f32), ALU.subtract)

                    for kc in range(NH):
                        gkc = half * NH + kc
                        ksl = slice(kc * 128, (kc + 1) * 128)
                        num_ps = ps_nd.tile([128, TS], f32, tag="num_ps")
                        nc.tensor.matmul(
                            num_ps[:], knr_hh[:, ksl], qnr_h[:],
                            start=True, stop=True)
                        den_ps = ps_nd.tile([128, TS], f32, tag="den_ps")
                        if DEN_SPLIT3:
                            nc.tensor.matmul(den_ps[:], Sr_hh[:, ksl], qnr_h[:],
                                             start=True, stop=False)
                            nc.tensor.matmul(den_ps[:], Sr_hh[:, ksl], qe_h[:],
                                             start=False, stop=False)
                            nc.tensor.matmul(den_ps[:], Se_hh[:, ksl], qnr_h[:],
                                             start=False, stop=True)
                        else:
                            nc.tensor.matmul(den_ps[:], S_hh[:, ksl], qn_h[:],
                                             start=True, stop=True)
                        rcp = ew.tile([128, TS], f32, tag="rcp")
                        if gkc < int(NKC * CLAMP_SPLIT):
                            denc = ew.tile([128, TS], f32, tag="denc")
                            nc.vector.tensor_scalar_max(
                                denc[:], den_ps[:], EPS_DENOM)
                            act_reciprocal(nc, rcp[:], denc[:])
                        else:
                            dsh = ew.tile([128, TS], f32, tag="dsh")
                            nc.scalar.activation(
                                dsh[:], den_ps[:], AF.Relu,
                                bias=negeps[:], scale=1.0)
                            act_reciprocal(nc, rcp[:], dsh[:], bias=EPS_DENOM)
                        att = ew.tile([128, TS], f32r, tag="att")
                        nc.vector.scalar_tensor_tensor(
                            att[:], num_ps[:], 1.0, rcp[:], ALU.mult, ALU.mult)
                        nc.tensor.matmul(
                            y_ps[:], v_h[:, gkc, :], att[:],
                            start=(gkc == 0), stop=(gkc == NKC - 1))
                ci, h2 = h // 2, h % 2
                nc.vector.tensor_copy(yT[ci][h2 * 64:(h2 + 1) * 64, :], y_ps[:])

            # output projection: out[t, c'] = y^T.T @ w_proj + b
            for tt in range(TS // 128):
                o_sb = ew.tile([128, C], f32, tag="o_sb")
                for c0, cn in ((0, 512), (512, 256)):
                    ps = ps_nd.tile([128, 512], f32, tag="o_ps")
                    for ci in range(NCH):
                        nc.tensor.matmul(
                            ps[:, :cn], yT[ci][:, tt * 128:(tt + 1) * 128],
                            wp_sb[ci][:, c0:c0 + cn],
                            start=(ci == 0), stop=False)
                    nc.tensor.matmul(
                        ps[:, :cn], ones_r[0:1, :], bp_sb[0:1, c0:c0 + cn],
                        start=False, stop=True)
                    nc.scalar.copy(o_sb[:, c0:c0 + cn], ps[:, :cn])
                nc.sync.dma_start(out_o[tt * 128:(tt + 1) * 128, :], o_sb[:])
    legalize_waits(nc)
    return nc


_built = {}


def _get(name, builder):
    if name not in _built:
        _built[name] = builder()
    return _built[name]


def run_launches(x, w_attn, b_attn, w_proj, b_proj, trace=False, trace_cores=None):
    xt_full = np.ascontiguousarray(x.reshape(T, C).T.astype(np.float32))  # [C, T]
    w_qk = np.ascontiguousarray(w_attn[:, :2 * C].astype(np.float32))
    w_v = np.ascontiguousarray(w_attn[:, 2 * C:].astype(np.float32))
    b_qk = np.ascontiguousarray(b_attn[:2 * C].astype(np.float32)).reshape(1, 2 * C)
    b_v = np.ascontiguousarray(b_attn[2 * C:].astype(np.float32)).reshape(1, C)

    nc1 = _get("l1", build_l1)
    in1 = [
        {
            "xT": np.ascontiguousarray(xt_full[:, i * TS:(i + 1) * TS]),
            "w_qk": w_qk, "w_v": w_v, "b_qk": b_qk, "b_v": b_v,
        }
        for i in range(N_CORES)
    ]
    kw = dict(trace=trace)
    if trace_cores is not None:
        kw["trace_cores"] = trace_cores
    r1 = run_bass_kernel_spmd(nc1, in1, core_ids=list(range(N_CORES)), **kw)

    kn = np.concatenate([r["kn_o"] for r in r1.results], axis=1)     # [C, T]
    knr = np.concatenate([r["knr_o"] for r in r1.results], axis=1)
    v_full = np.concatenate([r["v_o"] for r in r1.results], axis=0)  # [T, C]

    nc2 = _get("l2", build_l2)
    wp = np.ascontiguousarray(w_proj.astype(np.float32))
    bp = np.ascontiguousarray(b_proj.astype(np.float32)).reshape(1, C)
    in2 = [
        {
            "kn_i": kn, "knr_i": knr,
            "qn_i": r1.results[i]["qn_o"],
            "qnr_i": r1.results[i]["qnr_o"],
            "qe_i": r1.results[i]["qe_o"],
            "v_i": v_full, "w_proj": wp, "b_proj": bp,
        }
        for i in range(N_CORES)
    ]
    r2 = run_bass_kernel_spmd(nc2, in2, core_ids=list(range(N_CORES)), **kw)
    out = np.concatenate([r["out_o"] for r in r2.results], axis=0)
    return out.reshape(1, T, C), r1, r2


def kernel(x, w_attn, b_attn, w_proj, b_proj):
    out, _, _ = run_launches(
        np.asarray(x, dtype=np.float32),
        np.asarray(w_attn, dtype=np.float32),
        np.asarray(b_attn, dtype=np.float32),
        np.asarray(w_proj, dtype=np.float32),
        np.asarray(b_proj, dtype=np.float32),
    )
    return out.astype(np.float32)


# revision 8
# speedup vs baseline: 1.1214x; 1.1214x over previous
"""Trainium2 Bass kernel for nn_CausalSelfAttention_24034636988727 (B=1,T=4096,C=768,H=12).

Math identity used: denom = cumsum(qn@kn^T, axis=-1) = qn @ cumsum(kn, axis=0)^T,
so the TxT cumsum collapses to a [T,hd] prefix-sum plus a second matmul and the
whole attention stays on-chip (no TxT traffic to HBM).

Sharding (8 cores, two SPMD launches, full I/O in host numpy):
  L1: T-sharded qkv projection (q,k fp32; v->f32r), l2-normalize q,k,
      emit transposed [c',t] q,k plus f32r-rounded copies (and q residual for
      a 3-term f32r "split" den matmul at ~fp32 accuracy, 3 cyc/row vs 4).
  host: concatenate shards (data movement only).
  L2: q-block sharded. Per head: prefix-scan kn^T -> S (GPSIMD);
      num=qnr@knr^T (f32r, 1 cyc/row); den=Sr@qnr+Sr@qe+Se@qnr (f32r x3);
      att=num*recip(max(den,1e-6)) via DVE clamp + ACT reciprocal + DVE mult;
      y^T accumulated on PE (f32r); output projection (f32r) + biases.
"""

import sys

sys.path.insert(0, "/opt/trn_rl_repo")

import numpy as np

import concourse.bass as bass
import concourse.mybir as mybir
import concourse.tile as tile
from concourse.tile import ScopedClock
from concourse.bass_utils import run_bass_kernel_spmd

N_CORES = 8
T = 4096
C = 768
H = 12
HD = 64
TS = T // N_CORES        # 512 q rows per core
HALF = T // 2            # k-halves per head in L2 (SBUF footprint)
NKC = T // 128           # 32 k-chunks per head
NCH = C // 128           # 6 contraction chunks
f32 = mybir.dt.float32
f32r = mybir.dt.float32r
AF = mybir.ActivationFunctionType
ALU = mybir.AluOpType

EPS_NORM = 1e-12
EPS_DENOM = 1e-6

# tuning knobs
SCAN_ON_GPSIMD = False  # Pool TensorScalarPtr rejected by this walrus
DEN_SPLIT3 = True    # den via 3 f32r matmuls instead of 1 plain-fp32 matmul
CLAMP_SPLIT = 0.4    # fraction of k-chunks whose clamp runs on DVE (rest: ACT relu path)


class TC(tile.TileContext):
    """TileContext whose final drain spreads its waits over several SP drains
    (this walrus build allows only one sync wait per instruction)."""

    def _drain_and_barrier(self, tick_clock, wait_clock):
        nc = self.nc
        probe = nc.sync.drain()
        wait_clock.add_sem_waits(probe.ins, ScopedClock({None: tick_clock.global_clock}))
        waits = list(probe.ins.sync_info.on_wait)
        probe.ins.sync_info.on_wait = waits[:1]
        for w in waits[1:]:
            n2 = nc.sync.drain()
            si = n2.ins.sync_info
            if si is None:
                si = mybir.SyncInfo(on_wait=[], on_update=[])
                n2.ins.sync_info = si
            si.on_wait = [w]
        nc.all_engine_barrier()
        assert self.sems is not None
        popped = nc._tile_sem_poison_stack.pop()
        assert popped is self._sem_poison
        nc.clear_and_free_semaphores(list(self.sems.allocated().values()))
        nc.all_engine_barrier()


def legalize_waits(nc):
    """This walrus accepts at most one sync wait per instruction; hoist extra
    waits onto same-engine NoOps placed immediately before the instruction."""
    for f in nc.m.functions:
        for bb in f.blocks:
            out = []
            changed = False
            for ins in list(bb.instructions):
                si = ins.sync_info
                ow = list(si.on_wait) if (si is not None and si.on_wait) else []
                if len(ow) > 1:
                    for j, w in enumerate(ow[:-1]):
                        out.append(
                            mybir.InstNoOp(
                                name=f"{ins.name}-lw{j}",
                                engine=ins.engine,
                                ins=[],
                                outs=[],
                                sync_info=mybir.SyncInfo(on_wait=[w], on_update=[]),
                            )
                        )
                    si.on_wait = [ow[-1]]
                    ins.sync_info = si
                    changed = True
                out.append(ins)
            if changed:
                bb.instructions = out


def act_reciprocal(nc, out_ap, in_ap, bias=0.0):
    """1/(x+bias) on the Activation engine (direct emission; the bass wrapper
    blanket-bans Reciprocal, but measured accuracy here is ~1e-5 max rel err)."""
    return nc.scalar.add_instruction(
        mybir.InstActivation(
            name=nc.get_next_instruction_name(),
            func=AF.Reciprocal,
            ins=[
                nc.scalar.lower_ap(in_ap),
                mybir.ImmediateValue(dtype=f32, value=float(bias)),
                mybir.ImmediateValue(dtype=f32, value=1.0),
                mybir.ImmediateValue(dtype=f32, value=0.0),
            ],
            outs=[nc.scalar.lower_ap(out_ap)],
        )
    )


def build_l1():
    nc = bass.Bass("TRN2", target_bir_lowering=False, debug=False)
    xT = nc.dram_tensor("xT", [C, TS], f32, kind="ExternalInput")
    w_qk = nc.dram_tensor("w_qk", [C, 2 * C], f32, kind="ExternalInput")
    w_v = nc.dram_tensor("w_v", [C, C], f32, kind="ExternalInput")
    b_qk = nc.dram_tensor("b_qk", [1, 2 * C], f32, kind="ExternalInput")
    b_v = nc.dram_tensor("b_v", [1, C], f32, kind="ExternalInput")
    kn_o = nc.dram_tensor("kn_o", [C, TS], f32, kind="ExternalOutput")
    knr_o = nc.dram_tensor("knr_o", [C, TS], f32r, kind="ExternalOutput")
    qn_o = nc.dram_tensor("qn_o", [C, TS], f32, kind="ExternalOutput")
    qnr_o = nc.dram_tensor("qnr_o", [C, TS], f32r, kind="ExternalOutput")
    qe_o = nc.dram_tensor("qe_o", [C, TS], f32r, kind="ExternalOutput")
    v_o = nc.dram_tensor("v_o", [TS, C], f32r, kind="ExternalOutput")

    with TC(nc) as tc:
        with (
            tc.tile_pool(name="inp", bufs=1) as inp,
            tc.tile_pool(name="proj", bufs=1) as proj,
            tc.tile_pool(name="outw", bufs=3) as outw,
            tc.tile_pool(name="work", bufs=2) as work,
            tc.tile_pool(name="ps_a", bufs=2, space="PSUM") as ps_a,
            tc.tile_pool(name="ps_b", bufs=2, space="PSUM") as ps_b,
            tc.tile_pool(name="ps_c", bufs=2, space="PSUM") as ps_c,
        ):
            xt_sb = []
            for ci in range(NCH):
                t_ = inp.tile([128, TS], f32, tag=f"xt{ci}")
                nc.sync.dma_start(t_[:], xT[ci * 128:(ci + 1) * 128, :])
                xt_sb.append(t_)
            wqk_sb = []
            for ci in range(NCH):
                t_ = inp.tile([128, 2 * C], f32, tag=f"wqk{ci}")
                nc.sync.dma_start(t_[:], w_qk[ci * 128:(ci + 1) * 128, :])
                wqk_sb.append(t_)
            wv_sb = []
            for ci in range(NCH):
                t_ = inp.tile([128, C], f32, tag=f"wv{ci}")
                nc.sync.dma_start(t_[:], w_v[ci * 128:(ci + 1) * 128, :])
                wv_sb.append(t_)
            bqk_sb = inp.tile([1, 2 * C], f32, tag="bqk")
            nc.sync.dma_start(bqk_sb[:], b_qk[:])
            bv_sb = inp.tile([1, C], f32, tag="bv")
            nc.sync.dma_start(bv_sb[:], b_v[:])
            ones_r = inp.tile([12, TS], f32, tag="ones_r")
            nc.vector.memset(ones_r[:], 1.0)
            ones_c = inp.tile([128, 1], f32, tag="ones_c")
            nc.vector.memset(ones_c[:], 1.0)

            # q,k projection, transposed layout [c', t] (plain fp32 matmuls)
            qkT = []
            for j in range(12):
                ps = ps_a.tile([128, TS], f32, tag="proj_ps")
                for ci in range(NCH):
                    nc.tensor.matmul(
                        ps[:], wqk_sb[ci][:, j * 128:(j + 1) * 128], xt_sb[ci][:],
                        start=(ci == 0), stop=False)
                nc.tensor.matmul(
                    ps[:], bqk_sb[0:1, j * 128:(j + 1) * 128], ones_r[0:1, :],
                    start=False, stop=True)
                t_ = proj.tile([128, TS], f32, tag=f"qkT{j}")
                nc.scalar.copy(t_[:], ps[:])
                qkT.append(t_)

            # v projection, natural layout [t, c'] (fp32 matmul, f32r-rounded out)
            for tt in range(TS // 128):
                t_ = outw.tile([128, C], f32r, tag="v_nat")
                for c0, cn in ((0, 512), (512, 256)):
                    ps = ps_b.tile([128, 512], f32, tag="v_ps")
                    for ci in range(NCH):
                        nc.tensor.matmul(
                            ps[:, :cn],
                            xt_sb[ci][:, tt * 128:(tt + 1) * 128],
                            wv_sb[ci][:, c0:c0 + cn],
                            start=(ci == 0), stop=False)
                    nc.tensor.matmul(
                        ps[:, :cn], ones_r[0:1, 0:128], bv_sb[0:1, c0:c0 + cn],
                        start=False, stop=True)
                    nc.vector.tensor_copy(t_[:, c0:c0 + cn], ps[:, :cn])
                nc.sync.dma_start(v_o[tt * 128:(tt + 1) * 128, :], t_[:])

            # per-head l2 norms (sumsq over 64 partition rows via ones-matmul),
            # then normalize via ones-outer-product broadcast; round; residual.
            outs = {0: (qn_o, qnr_o), 1: (kn_o, knr_o)}
            for qk in range(2):  # 0: q, 1: k
                o_f32, o_f32r = outs[qk]
                for j in range(6):
                    sq = work.tile([128, TS], f32, tag="sq")
                    nc.scalar.square(sq[:], qkT[qk * 6 + j][:])
                    nrm_t = outw.tile([128, TS], f32, tag="nrmd")
                    rnd_t = outw.tile([128, TS], f32r, tag="rndd")
                    for h2 in range(2):
                        ps1 = ps_c.tile([1, TS], f32, tag="red_ps")
                        nc.tensor.matmul(
                            ps1[:], ones_c[h2 * 64:(h2 + 1) * 64, :],
                            sq[h2 * 64:(h2 + 1) * 64, :], start=True, stop=True)
                        sn = work.tile([1, TS], f32, tag="sn")
                        nc.scalar.sqrt(sn[:], ps1[:])
                        nc.vector.tensor_scalar_max(sn[:], sn[:], EPS_NORM)
                        rn = work.tile([1, TS], f32, tag="rn")
                        act_reciprocal(nc, rn[:], sn[:])
                        psb = ps_c.tile([64, TS], f32, tag="bcast_ps")
                        nc.tensor.matmul(
                            psb[:], ones_r[0:1, 0:64], rn[:],
                            start=True, stop=True)
                        nc.vector.scalar_tensor_tensor(
                            nrm_t[h2 * 64:(h2 + 1) * 64, :], psb[:], 1.0,
                            qkT[qk * 6 + j][h2 * 64:(h2 + 1) * 64, :],
                            ALU.mult, ALU.mult)
                    nc.vector.tensor_copy(rnd_t[:], nrm_t[:])
                    nc.sync.dma_start(o_f32[j * 128:(j + 1) * 128, :], nrm_t[:])
                    nc.sync.dma_start(o_f32r[j * 128:(j + 1) * 128, :], rnd_t[:])
                    if qk == 0 and DEN_SPLIT3:
                        qe_t = outw.tile([128, TS], f32r, tag="qe")
                        nc.vector.tensor_tensor(
                            qe_t[:], nrm_t[:], rnd_t[:].bitcast(f32), ALU.subtract)
                        nc.sync.dma_start(qe_o[j * 128:(j + 1) * 128, :], qe_t[:])
    legalize_waits(nc)
    return nc


def build_l2():
    nc = bass.Bass("TRN2", target_bir_lowering=False, debug=False)
    kn_i = nc.dram_tensor("kn_i", [C, T], f32, kind="ExternalInput")
    knr_i = nc.dram_tensor("knr_i", [C, T], f32r, kind="ExternalInput")
    qn_i = nc.dram_tensor("qn_i", [C, TS], f32, kind="ExternalInput")
    qnr_i = nc.dram_tensor("qnr_i", [C, TS], f32r, kind="ExternalInput")
    qe_i = nc.dram_tensor("qe_i", [C, TS], f32r, kind="ExternalInput")
    v_i = nc.dram_tensor("v_i", [T, C], f32r, kind="ExternalInput")
    w_proj = nc.dram_tensor("w_proj", [C, C], f32, kind="ExternalInput")
    b_proj = nc.dram_tensor("b_proj", [1, C], f32, kind="ExternalInput")
    out_o = nc.dram_tensor("out_o", [TS, C], f32, kind="ExternalOutput")

    NH = HALF // 128  # 16 k-chunks per half

    with TC(nc) as tc:
        with (
            tc.tile_pool(name="inp", bufs=1) as inp,
            tc.tile_pool(name="qh", bufs=2) as qh,
            tc.tile_pool(name="kh", bufs=2) as kh,
            tc.tile_pool(name="ew", bufs=3) as ew,
            tc.tile_pool(name="ps_nd", bufs=2, space="PSUM") as ps_nd,
            tc.tile_pool(name="ps_y", bufs=2, space="PSUM") as ps_y,
        ):
            ones_r = inp.tile([1, 128], f32, tag="ones_r")
            nc.vector.memset(ones_r[:], 1.0)
            negeps = inp.tile([128, 1], f32, tag="negeps")
            nc.vector.memset(negeps[:], -EPS_DENOM)
            wp_sb = []
            for ci in range(NCH):
                tf_ = inp.tile([128, C], f32, tag="wp_tmp")
                nc.sync.dma_start(tf_[:], w_proj[ci * 128:(ci + 1) * 128, :])
                wr = inp.tile([128, C], f32r, tag=f"wpr{ci}")
                nc.vector.tensor_copy(wr[:], tf_[:])
                wp_sb.append(wr)
            bp_sb = inp.tile([1, C], f32, tag="bp")
            nc.sync.dma_start(bp_sb[:], b_proj[:])
            yT = []
            for ci in range(NCH):
                yt_t = inp.tile([128, TS], f32r, tag=f"yT{ci}")
                yT.append(yt_t)

            for h in range(H):
                hs = slice(h * 64, (h + 1) * 64)
                qnr_h = qh.tile([64, TS], f32r, tag="qnr_h")
                nc.sync.dma_start(qnr_h[:], qnr_i[hs, :])
                if DEN_SPLIT3:
                    qe_h = qh.tile([64, TS], f32r, tag="qe_h")
                    nc.sync.dma_start(qe_h[:], qe_i[hs, :])
                else:
                    qn_h = qh.tile([64, TS], f32, tag="qn_h")
                    nc.sync.dma_start(qn_h[:], qn_i[hs, :])
                v_h = qh.tile([128, NKC, 64], f32r, tag="v_h")
                nc.sync.dma_start(
                    v_h[:], v_i[:, hs].rearrange("(c p) d -> p c d", p=128))

                y_ps = ps_y.tile([64, TS], f32, tag="y_ps")
                prev_S = None
                for half in range(2):
                    hsl = slice(half * HALF, (half + 1) * HALF)
                    kn_hh = kh.tile([64, HALF], f32, tag="kn_h")
                    nc.sync.dma_start(kn_hh[:], kn_i[hs, hsl])
                    knr_hh = kh.tile([64, HALF], f32r, tag="knr_h")
                    nc.sync.dma_start(knr_hh[:], knr_i[hs, hsl])
                    S_hh = kh.tile([64, HALF], f32, tag="S_h")
                    init = 0.0 if half == 0 else prev_S[:, HALF - 1:HALF]
                    eng = nc.gpsimd if SCAN_ON_GPSIMD else nc.vector
                    eng.tensor_tensor_scan(
                        S_hh[:], kn_hh[:], kn_hh[:], init, ALU.add, ALU.bypass)
                    prev_S = S_hh
                    if DEN_SPLIT3:
                        Sr_hh = kh.tile([64, HALF], f32r, tag="Sr_h")
                        nc.scalar.copy(Sr_hh[:], S_hh[:])
                        Se_hh = kh.tile([64, HALF], f32r, tag="Se_h")
                        nc.vector.tensor_tensor(
                            Se_hh[:], S_hh[:], Sr_hh[:].bitcast(f32), ALU.subtract)

                    for kc in range(NH):
                        gkc = half * NH + kc
                        ksl = slice(kc * 128, (kc + 1) * 128)
                        num_ps = ps_nd.tile([128, TS], f32, tag="num_ps")
                        nc.tensor.matmul(
                            num_ps[:], knr_hh[:, ksl], qnr_h[:],
                            start=True, stop=True)
                        den_ps = ps_nd.tile([128, TS], f32, tag="den_ps")
                        if DEN_SPLIT3:
                            nc.tensor.matmul(den_ps[:], Sr_hh[:, ksl], qnr_h[:],
                                             start=True, stop=False)
                            nc.tensor.matmul(den_ps[:], Sr_hh[:, ksl], qe_h[:],
                                             start=False, stop=False)
                            nc.tensor.matmul(den_ps[:], Se_hh[:, ksl], qnr_h[:],
                                             start=False, stop=True)
                        else:
                            nc.tensor.matmul(den_ps[:], S_hh[:, ksl], qn_h[:],
                                             start=True, stop=True)
                        rcp = ew.tile([128, TS], f32, tag="rcp")
                        if gkc < int(NKC * CLAMP_SPLIT):
                            denc = ew.tile([128, TS], f32, tag="denc")
                            nc.vector.tensor_scalar_max(
                                denc[:], den_ps[:], EPS_DENOM)
                            act_reciprocal(nc, rcp[:], denc[:])
                        else:
                            dsh = ew.tile([128, TS], f32, tag="dsh")
                            nc.scalar.activation(
                                dsh[:], den_ps[:], AF.Relu,
                                bias=negeps[:], scale=1.0)
                            act_reciprocal(nc, rcp[:], dsh[:], bias=EPS_DENOM)
                        att = ew.tile([128, TS], f32r, tag="att")
                        nc.vector.scalar_tensor_tensor(
                            att[:], num_ps[:], 1.0, rcp[:], ALU.mult, ALU.mult)
                        nc.tensor.matmul(
                            y_ps[:], v_h[:, gkc, :], att[:],
                            start=(gkc == 0), stop=(gkc == NKC - 1))
                ci, h2 = h // 2, h % 2
                nc.vector.tensor_copy(yT[ci][h2 * 64:(h2 + 1) * 64, :], y_ps[:])

            # output projection: out[t, c'] = y^T.T @ w_proj + b
            for tt in range(TS // 128):
                o_sb = ew.tile([128, C], f32, tag="o_sb")
                for c0, cn in ((0, 512), (512, 256)):
                    ps = ps_nd.tile([128, 512], f32, tag="o_ps")
                    for ci in range(NCH):
                        nc.tensor.matmul(
                            ps[:, :cn], yT[ci][:, tt * 128:(tt + 1) * 128],
                            wp_sb[ci][:, c0:c0 + cn],
                            start=(ci == 0), stop=False)
                    nc.tensor.matmul(
                        ps[:, :cn], ones_r[0:1, :], bp_sb[0:1, c0:c0 + cn],
                        start=False, stop=True)
                    nc.scalar.copy(o_sb[:, c0:c0 + cn], ps[:, :cn])
                nc.sync.dma_start(out_o[tt * 128:(tt + 1) * 128, :], o_sb[:])
    legalize_waits(nc)
    return nc


_built = {}


def _get(name, builder):
    if name not in _built:
        _built[name] = builder()
    return _built[name]


def run_launches(x, w_attn, b_attn, w_proj, b_proj, trace=False, trace_cores=None):
    xt_full = np.ascontiguousarray(x.reshape(T, C).T.astype(np.float32))  # [C, T]
    w_qk = np.ascontiguousarray(w_attn[:, :2 * C].astype(np.float32))
    w_v = np.ascontiguousarray(w_attn[:, 2 * C:].astype(np.float32))
    b_qk = np.ascontiguousarray(b_attn[:2 * C].astype(np.float32)).reshape(1, 2 * C)
    b_v = np.ascontiguousarray(b_attn[2 * C:].astype(np.float32)).reshape(1, C)

    nc1 = _get("l1", build_l1)
    in1 = [
        {
            "xT": np.ascontiguousarray(xt_full[:, i * TS:(i + 1) * TS]),
            "w_qk": w_qk, "w_v": w_v, "b_qk": b_qk, "b_v": b_v,
        }
        for i in range(N_CORES)
    ]
    kw = dict(trace=trace)
    if trace_cores is not None:
        kw["trace_cores"] = trace_cores
    r1 = run_bass_kernel_spmd(nc1, in1, core_ids=list(range(N_CORES)), **kw)

    kn = np.concatenate([r["kn_o"] for r in r1.results], axis=1)     # [C, T]
    knr = np.concatenate([r["knr_o"] for r in r1.results], axis=1)
    v_full = np.concatenate([r["v_o"] for r in r1.results], axis=0)  # [T, C]

    nc2 = _get("l2", build_l2)
    wp = np.ascontiguousarray(w_proj.astype(np.float32))
    bp = np.ascontiguousarray(b_proj.astype(np.float32)).reshape(1, C)
    in2 = [
        {
            "kn_i": kn, "knr_i": knr,
            "qn_i": r1.results[i]["qn_o"],
            "qnr_i": r1.results[i]["qnr_o"],
            "qe_i": r1.results[i]["qe_o"],
            "v_i": v_full, "w_proj": wp, "b_proj": bp,
        }
        for i in range(N_CORES)
    ]
    r2 = run_bass_kernel_spmd(nc2, in2, core_ids=list(range(N_CORES)), **kw)
    out = np.concatenate([r["out_o"] for r in r2.results], axis=0)
    return out.reshape(1, T, C), r1, r2


def kernel(x, w_attn, b_attn, w_proj, b_proj):
    out, _, _ = run_launches(
        np.asarray(x, dtype=np.float32),
        np.asarray(w_attn, dtype=np.float32),
        np.asarray(b_attn, dtype=np.float32),
        np.asarray(w_proj, dtype=np.float32),
        np.asarray(b_proj, dtype=np.float32),
    )
    return out.astype(np.float32)
